# revision 1
# baseline (speedup 1.0000x reference)
"""CapsuleLayer Trainium2 kernel.

Per-core work (data-parallel over batch N=8 -> 8 cores):
  x_i [t0=8, z0=32, 64, 64] -> conv(stride2,pad1,3x3, 512ch) -> u [t0, (z1,t1), 32x32]
  3 dynamic-routing iterations -> v [t1=8, z1=64, 32, 32]

Layout choices:
  - conv matmuls: lhsT = shifted x windows [K=97=(3kh*32z0 + ones), hw-chunk 128],
    rhs = host-repacked weights [97, 512=(z1*8+t1)], psum out [hw 128, 512].
    So u lands directly in "hw on partitions" routing layout, fp16 in SBUF.
  - routing einsums on DVE in fp16: multiplies via broadcast APs (2x mode),
    reductions via strided tree-adds.
  - softmax/maxpool on a transposed [64=(t0,t1), 1024=hw] view (PE transposes).
  - iteration-1 softmax is uniform => p1 = (sum_t0 u)/8 (no r needed).
"""

import numpy as np

import concourse.bass as bass
import concourse.tile as tile
from concourse import mybir
from concourse.bass_utils import run_bass_kernel_spmd

F32 = mybir.dt.float32
F32R = mybir.dt.float32r
F16 = mybir.dt.float16
BF16 = mybir.dt.bfloat16

T0, T1, Z1 = 8, 8, 64
H1 = W1 = 32
HW = H1 * W1          # 1024
NCHUNK = 8            # hw chunks of 128 partitions = 4 oh rows each
K = 97                # 3*32 + ones row
EPS = 1e-9
CLAMP = 60.0

AF = mybir.ActivationFunctionType
ALU = mybir.AluOpType

_MAXW = 1


def _split_waits(nc):
    """walrus in this container rejects >1 sync wait per instruction; hoist
    excess waits onto preceding NoOps on the same engine."""
    for fn in nc.m.functions:
        for blk in fn.blocks:
            new_insts = []
            for ins in blk.instructions:
                si = ins.sync_info
                if si is not None and len(si.on_wait) > _MAXW:
                    waits = list(si.on_wait)
                    extra, keep = waits[:-_MAXW], waits[-_MAXW:]
                    for i in range(0, len(extra), _MAXW):
                        new_insts.append(
                            mybir.InstNoOp(
                                name=f"{ins.name}-wsplit{i}",
                                engine=ins.engine,
                                sync_info=mybir.SyncInfo(
                                    on_wait=extra[i : i + _MAXW], on_update=[]
                                ),
                            )
                        )
                    si.on_wait = keep
                new_insts.append(ins)
            blk.instructions = new_insts
    return nc


def _squash_scale(nc, n2raw, sc, pre, post, tmp_pool, w):
    """Given n2raw[128,w] (= sum_z1 p^2 with p the UNSCALED accumulator),
    write sc[128,w] fp32 so that  v = squash(pre*p) = p * sc.
    sc = pre^3*n2raw / ((1 + pre^2*n2raw) * sqrt(pre^2*n2raw + EPS)) * post
    (post lets callers fold extra constant factors in)."""
    p2 = pre * pre
    t1 = tmp_pool.tile([128, w], F32, name="sqt1", tag="sqt1")
    # t1 = n2*pre^2 + 1
    nc.vector.tensor_scalar(t1[:], n2raw[:], p2, 1.0, op0=ALU.mult, op1=ALU.add)
    t2 = tmp_pool.tile([128, w], F32, name="sqt2", tag="sqt2")
    # t2 = n2*pre^2 + EPS
    nc.vector.tensor_scalar(t2[:], n2raw[:], p2, EPS, op0=ALU.mult, op1=ALU.add)
    t3 = tmp_pool.tile([128, w], F32, name="sqt3", tag="sqt3")
    nc.scalar.activation(t3[:], t2[:], AF.Sqrt)
    t4 = tmp_pool.tile([128, w], F32, name="sqt4", tag="sqt4")
    nc.vector.tensor_tensor(t4[:], t1[:], t3[:], op=ALU.mult)
    t5 = tmp_pool.tile([128, w], F32, name="sqt5", tag="sqt5")
    nc.vector.reciprocal(t5[:], t4[:])
    t6 = tmp_pool.tile([128, w], F32, name="sqt6", tag="sqt6")
    nc.vector.tensor_tensor(t6[:], t5[:], n2raw[:], op=ALU.mult)
    nc.vector.tensor_scalar_mul(sc[:], t6[:], p2 * pre * post)


def build_module(split=True, phases=99):
    nc = bass.Bass("TRN2", target_bir_lowering=False, debug=False)

    x = nc.dram_tensor("x", [T0, 32, 64, 64], F32, kind="ExternalInput")
    wt = nc.dram_tensor("wt", [3, K, 512], F32R, kind="ExternalInput")
    ident = nc.dram_tensor("ident", [128, 128], F16, kind="ExternalInput")
    ind8 = nc.dram_tensor("ind8", [64, T0], BF16, kind="ExternalInput")
    ind8t = nc.dram_tensor("ind8t", [T0, 64], F32, kind="ExternalInput")
    out = nc.dram_tensor("out", [T1, Z1, H1, W1], F32, kind="ExternalOutput")
    out_f = out.ap().rearrange("a b c d -> (a b c d)")

    with tile.TileContext(nc) as tc:
        import contextlib

        with contextlib.ExitStack() as ctx:
            consts = ctx.enter_context(tc.tile_pool(name="consts", bufs=1))
            ypool = ctx.enter_context(tc.tile_pool(name="ypool", bufs=1))
            upool = ctx.enter_context(tc.tile_pool(name="upool", bufs=1))
            tree = ctx.enter_context(tc.tile_pool(name="tree", bufs=3))
            small = ctx.enter_context(tc.tile_pool(name="small", bufs=3))
            persm = ctx.enter_context(tc.tile_pool(name="persm", bufs=1))
            rphase = ctx.enter_context(tc.tile_pool(name="rphase", bufs=1))
            vout = ctx.enter_context(tc.tile_pool(name="vout", bufs=1))
            ps_conv = ctx.enter_context(
                tc.tile_pool(name="ps_conv", bufs=2, space="PSUM")
            )
            ps_t = ctx.enter_context(tc.tile_pool(name="ps_t", bufs=2, space="PSUM"))
            ps_p1 = ctx.enter_context(tc.tile_pool(name="ps_p1", bufs=1, space="PSUM"))
            ps_s = ctx.enter_context(tc.tile_pool(name="ps_s", bufs=1, space="PSUM"))
            ps_sb = ctx.enter_context(tc.tile_pool(name="ps_sb", bufs=1, space="PSUM"))

            # ---------------- constants ----------------
            wt_sb = [
                consts.tile([K, 512], F32R, name=f"wt{kw}", tag=f"wt{kw}")
                for kw in range(3)
            ]
            for kw in range(3):
                nc.sync.dma_start(wt_sb[kw][:], wt.ap()[kw])
            id_sb = consts.tile([128, 128], F16, name="ident", tag="ident")
            nc.sync.dma_start(id_sb[:], ident.ap())
            ind8_sb = consts.tile([64, T0], BF16, name="ind8", tag="ind8")
            nc.sync.dma_start(ind8_sb[:], ind8.ap())
            ind8t_sb = consts.tile([T0, 64], F32, name="ind8t", tag="ind8t")
            nc.sync.dma_start(ind8t_sb[:], ind8t.ap())

            # ---------------- x staging -------
            # y_raw[(kh,z0), oh=32, c=66]; y_raw[p, oh, c] = x[z0, 2oh+kh-1, c-1]
            # y3[(kh,z0)+ones, kw-plane=3, oh=32, ow=32]:
            #   y3[p, kw, oh, ow] = x_pad[z0, 2oh+kh, 2ow+kw]  (single-stride
            #   windows so the matmul stationary operand has one free dim)
            # yraw2: x replicated to the 3 kh partition blocks (one fat DMA
            # with 16KB/partition descriptors -- the strided row/col selection
            # is done by on-chip engine copies instead of DMA descriptors).
            yraw2b = []
            for i in range(2):
                yr = ypool.tile([96, 64, 64], F32, name=f"yraw{i}", tag=f"yraw{i}")
                nc.vector.memset(yr[0:32, 0:1, :], 0.0)  # defined t0=0 garbage row
                yraw2b.append(yr)
            ybufs = []
            for i in range(2):
                y = ypool.tile([K, 3, 32, 32], F32R, name=f"y{i}", tag=f"y{i}")
                nc.vector.memset(y[96:97, :, :, :].bitcast(F32), 1.0)  # ones (bias)
                nc.vector.memset(y[0:96, 0, :, 0:1].bitcast(F32), 0.0)  # w=-1 pad
                nc.vector.memset(y[0:32, :, 0:1, :].bitcast(F32), 0.0)  # h=-1 pad
                ybufs.append(y)

            xa = x.ap()
            _cpeng = [nc.vector.tensor_copy, nc.scalar.copy, nc.gpsimd.tensor_copy]

            def load_y(t0):
                # Replicate x into 3 kh blocks PRE-SHIFTED by kh-1 rows, so
                # the deinterleave copies use uniform row APs on 96 partitions.
                # yraw2[(kh,z0), r, w] = x[t0, z0, r+kh-1, w].
                yraw2 = yraw2b[t0 % 2]
                if t0 == 0:
                    nc.sync.dma_start(yraw2[0:32, 1:64, :], xa[0, :, 0:63, :])
                    src = bass.AP(
                        tensor=xa.tensor,
                        offset=0,
                        ap=[[64, 2], [4096, 32], [1, 4032]],
                    )
                    nc.sync.dma_start(
                        yraw2[32:96, 0:63, :].rearrange("p a b -> p (a b)"), src
                    )
                else:
                    src = bass.AP(
                        tensor=xa.tensor,
                        offset=t0 * 32 * 4096 - 64,
                        ap=[[64, 3], [4096, 32], [1, 4032]],
                    )
                    nc.sync.dma_start(
                        yraw2[:, 0:63, :].rearrange("p a b -> p (a b)"), src
                    )
                y = ybufs[t0 % len(ybufs)]
                # 3 plane copies (rows 2oh uniform, cols 2ow+pl-1); then re-zero
                # the h=-1 pad row (kh=0 block oh=0) the copies clobbered.
                for pl in range(3):
                    ow0, col0, n_ow = (1, 1, 31) if pl == 0 else (0, pl - 1, 32)
                    src_v = yraw2[0:96, 0:63:2, col0 : col0 + 2 * n_ow - 1 : 2]
                    dst_v = y[0:96, pl, 0:32, ow0 : ow0 + n_ow]
                    _cpeng[pl](dst_v, src_v)
                nc.gpsimd.memset(y[0:32, :, 0:1, :].bitcast(F32), 0.0)
                return y

            # persistent routing state
            b = [persm.tile([128, T0 * T1], F16, name=f"b{c}", tag=f"b{c}") for c in range(NCHUNK)]
            r = [persm.tile([128, T0, T1], F16, name=f"r{c}", tag=f"r{c}") for c in range(NCHUNK)]
            # small staging for transposed output blocks (DMA'd out per (c,j))

            def t0_sum_pe(src, p_sb, pool=None, tag="pu"):
                """p_sb[128,512] f16 <- sum_t0 src[128, T0, 512] via identity
                matmuls accumulating in PSUM (PE is idle during routing)."""
                pp = (pool or ps_conv).tile([128, 512], F32, name=tag, tag=tag)
                for t0 in range(T0):
                    nc.tensor.matmul(
                        pp[:],
                        id_sb[:],
                        src[:, t0, :],
                        start=(t0 == 0),
                        stop=(t0 == T0 - 1),
                    )
                nc.scalar.copy(p_sb[:], pp[:])
                return p_sb

            def z1_sum(c, prod, first):
                """b[c] (+)= sum_z1 prod[128, T0, Z1, T1]: two DVE tree levels,
                then 16 identity-matmuls accumulate the tail on PE."""
                l1 = tree.tile([128, T0, 32, T1], F16, name="zt0", tag="zt0", bufs=2)
                s = prod.rearrange("p a (zh two) b -> p a zh two b", two=2)
                nc.vector.tensor_tensor(
                    l1[:], s[:, :, :, 0, :], s[:, :, :, 1, :], op=ALU.add
                )
                pz = ps_conv.tile([128, T0 * T1], F32, name="pu", tag="pu")
                for z in range(32):
                    nc.tensor.matmul(
                        pz.rearrange("p (a b) -> p a b", b=T1),
                        id_sb[:],
                        l1[:, :, z, :],
                        start=(z == 0),
                        stop=(z == 31),
                    )
                if first:
                    nc.scalar.copy(b[c][:], pz[:])
                else:
                    nc.vector.tensor_tensor(b[c][:], b[c][:], pz[:], op=ALU.add)

            def squash_one(c, p_acc, pre, v_tile):
                n2 = small.tile([128, T1], F32, name="sqn2", tag="sqn2", bufs=4)
                squash_n2(c, p_acc, n2, single=True)
                sc = small.tile([128, T1], F32, name="sqsc", tag="sqsc", bufs=4)
                _squash_scale(nc, n2, sc, pre, 1.0, small, T1)
                squash_v(c, p_acc, sc, v_tile, single=True)

            def squash_n2(c, p_acc, n2_all, single=False):
                """n2_all[:, 8c:8c+8] <- sum_z1 p^2 per t1."""
                pz = p_acc.rearrange("p (z t) -> p z t", t=T1)
                sq = small.tile([128, Z1, T1], F16, name="sqsq", tag="sqsq", bufs=4)
                nc.gpsimd.tensor_tensor(sq[:], pz[:], pz[:], op=ALU.mult)
                dst = n2_all[:] if single else n2_all[:, T1 * c : T1 * (c + 1)]
                nc.vector.tensor_reduce(
                    dst,
                    sq.transpose([0, 2, 1]),
                    axis=mybir.AxisListType.X,
                    op=ALU.add,
                )

            def squash_v(c, p_acc, sc_all, v_tile, single=False):
                pz = p_acc.rearrange("p (z t) -> p z t", t=T1)
                scs = sc_all[:] if single else sc_all[:, T1 * c : T1 * (c + 1)]
                scb = scs.unsqueeze(1).broadcast_to([128, Z1, T1])
                eng = nc.gpsimd
                eng.tensor_tensor(
                    v_tile.rearrange("p (z t) -> p z t", t=T1), pz[:], scb, op=ALU.mult
                )

            def b_einsum(c, v_tile, first):
                """b[c] (+)= sum_z1 u * v  (v broadcast over t0)."""
                prod = tree.tile([128, T0, Z1 * T1], F16, name="trP", tag="trP", bufs=2)
                vb = (
                    v_tile.unsqueeze(1).broadcast_to([128, T0, Z1 * T1])
                )
                nc.vector.tensor_tensor(prod[:], U[c][:], vb, op=ALU.mult)
                z1_sum(c, prod.rearrange("p a (z b) -> p a z b", b=T1), first)

            def r_phase():
                """r[c] <- softmax_t1(maxpool3x3(b)) for all chunks."""
                bT = rphase.tile([64, H1, W1], F16, name="bT", tag="bT")
                for c in range(NCHUNK):
                    pt = ps_t.tile([128, 128], F16, name="ptr", tag="ptr")
                    nc.tensor.transpose(pt[0:64, :], b[c][:], id_sb[:])
                    nc.scalar.copy(
                        bT[:, 4 * c : 4 * c + 4, :].rearrange("p a b -> p (a b)"),
                        pt[0:64, :],
                    )
                # maxpool: w direction then h direction
                mw = rphase.tile([64, H1, W1], F16, name="mw", tag="mw", bufs=1)
                nc.vector.tensor_tensor(
                    mw[:, :, 0:31], bT[:, :, 0:31], bT[:, :, 1:32], op=ALU.max
                )
                nc.gpsimd.tensor_copy(mw[:, :, 31:32], bT[:, :, 31:32])
                nc.vector.tensor_tensor(
                    mw[:, :, 1:32], mw[:, :, 1:32], bT[:, :, 0:31], op=ALU.max
                )
                mp = rphase.tile([64, H1, W1], F16, name="mp", tag="mp", bufs=1)
                nc.vector.tensor_tensor(
                    mp[:, 0:31, :], mw[:, 0:31, :], mw[:, 1:32, :], op=ALU.max
                )
                nc.gpsimd.tensor_copy(mp[:, 31:32, :], mw[:, 31:32, :])
                nc.vector.tensor_tensor(
                    mp[:, 1:32, :], mp[:, 1:32, :], mw[:, 0:31, :], op=ALU.max
                )
                nc.vector.tensor_scalar_min(mp[:], mp[:], CLAMP)
                # E = exp(mp)  (bf16: range safety)
                E = rphase.tile([64, HW], BF16, name="E", tag="E")
                nc.scalar.activation(
                    E.rearrange("p (a b) -> p a b", b=W1), mp[:], AF.Exp
                )
                # S = sum_t1 E : [8, 1024] psum
                S = ps_s.tile([T0, HW], F32, name="S", tag="S")
                for h in range(2):
                    nc.tensor.matmul(
                        S[:, 512 * h : 512 * (h + 1)],
                        ind8_sb[:],
                        E[:, 512 * h : 512 * (h + 1)],
                        start=True,
                        stop=True,
                    )
                Sr = rphase.tile([T0, HW], F32, name="Sr", tag="Sr", bufs=1)
                nc.vector.reciprocal(Sr[:], S[:])
                rT = rphase.tile([64, HW], F16, name="rT", tag="rT")
                for h in range(2):
                    Sb = ps_sb.tile([64, 512], F32, name="Sb", tag="Sb")
                    nc.tensor.matmul(
                        Sb[:],
                        ind8t_sb[:],
                        Sr[:, 512 * h : 512 * (h + 1)],
                        start=True,
                        stop=True,
                    )
                    nc.vector.tensor_tensor(
                        rT[:, 512 * h : 512 * (h + 1)],
                        E[:, 512 * h : 512 * (h + 1)],
                        Sb[:],
                        op=ALU.mult,
                    )
                # transpose back per chunk -> r[c] [128, (t0,t1)]
                rTv = rT.rearrange("p (a b) -> p a b", b=W1)
                for c in range(NCHUNK):
                    pt = ps_t.tile([128, 128], F16, name="ptr", tag="ptr")
                    nc.tensor.transpose(
                        pt[:, 0:64],
                        rTv[:, 4 * c : 4 * c + 4, :].rearrange("p a b -> p (a b)"),
                        id_sb[0:64, 0:64],
                    )
                    nc.scalar.copy(r[c].rearrange("p a b -> p (a b)"), pt[:, 0:64])

            def p_einsum(c, p_tile):
                """p = sum_t0 r * u  : [128, 512] f16."""
                prod = tree.tile([128, T0, 512], F16, name="trP", tag="trP", bufs=2)
                rb = (
                    r[c]
                    .unsqueeze(2)
                    .broadcast_to([128, T0, Z1, T1])
                )
                nc.vector.tensor_tensor(
                    prod.rearrange("p a (z b) -> p a z b", b=T1), U[c][:], rb,
                    op=ALU.mult,
                )
                t0_sum_pe(prod[:], p_tile)

            vT = [
                vout.tile([128, HW], F32, name=f"vT{j}", tag=f"vT{j}")
                for j in range(4)
            ]

            def out_chunk(c, v_tile):
                """transpose v [128hw, 512] -> vT[j][:, 128c:] fp32 staging."""
                for j in range(4):
                    pt = ps_t.tile([128, 128], F16, name="ptr", tag="ptr")
                    nc.tensor.transpose(
                        pt[:], v_tile[:, 128 * j : 128 * (j + 1)], id_sb[:]
                    )
                    nc.scalar.copy(vT[j][:, 128 * c : 128 * (c + 1)], pt[:])

            # ---------------- conv ----------------
            U = [upool.tile([128, T0, 512], F16, name=f"U{c}", tag=f"U{c}") for c in range(NCHUNK)]
            ps1 = [
                small.tile([128, 512], F16, name=f"pacc{c}", tag=f"pacc{c}", bufs=1)
                for c in range(NCHUNK)
            ]
            for t0 in range(T0):
                y = load_y(t0)
                for c in range(NCHUNK):
                    pu = ps_conv.tile([128, 512], F32, name="pu", tag="pu")
                    yv = y.rearrange("p k a b -> p k (a b)")
                    for kw in range(3):
                        nc.tensor.matmul(
                            pu[:],
                            yv[:, kw, 128 * c : 128 * (c + 1)],
                            wt_sb[kw][:],
                            start=(kw == 0),
                            stop=(kw == 2),
                        )
                    if (t0 + c) % 2 == 0:
                        nc.scalar.copy(U[c][:, t0, :], pu[:])
                    else:
                        nc.vector.tensor_copy(U[c][:, t0, :], pu[:])
                    if t0 == T0 - 1 and phases >= 1:
                        t0_sum_pe(U[c][:], ps1[c], pool=ps_p1, tag="pp1")

            # ---------------- routing ----------------
            # Stage-batched loops: all chunks' stage-k ops are emitted together
            # so each engine always has independent work from other chunks.
            # iter 1: r uniform=1/8 -> p1 = (sum_t0 u)/8; v1 = squash(p1)
            if phases >= 1:
                vs = [
                    small.tile([128, 512], F16, name=f"vt{c}", tag=f"vt{c}", bufs=1)
                    for c in range(NCHUNK)
                ]
                for c in range(NCHUNK):
                    squash_one(c, ps1[c], 1.0 / T0, vs[c])
                for c in range(NCHUNK):
                    b_einsum(c, vs[c], first=True)

            # iters 2..3
            for it in (2, 3):
                if it > phases:
                    break
                r_phase()
                last = it == 3
                ps = [
                    small.tile([128, 512], F16, name=f"pacc{c}", tag=f"pacc{c}", bufs=1)
                    for c in range(NCHUNK)
                ]
                vs = [
                    small.tile([128, 512], F16, name=f"vt{c}", tag=f"vt{c}", bufs=1)
                    for c in range(NCHUNK)
                ]
                for c in range(NCHUNK):
                    p_einsum(c, ps[c])
                for c in range(NCHUNK):
                    squash_one(c, ps[c], 1.0, vs[c])
                for c in range(NCHUNK):
                    if not last:
                        b_einsum(c, vs[c], first=False)
                    else:
                        out_chunk(c, vs[c])

            # vT[j] partitions p=(z1l*8+t1), z1=j*16+z1l -> out row t1*64+z1
            for j in range(4 if phases >= 3 else 0):
                dst = bass.AP(
                    tensor=out_f.tensor,
                    offset=j * 16 * HW,
                    ap=[[HW, 16], [64 * HW, 8], [1, HW]],
                )
                nc.sync.dma_start(dst, vT[j][:])

    return _split_waits(nc) if split else nc


# ---------------------------------------------------------------------------
_NC = None


def _get_nc(split=True):
    global _NC
    if _NC is None:
        _NC = build_module(split)
    return _NC


def _host_prep(W, bias):
    # wt[kw][kh*32+z0, z1*8+t1] = W[t1*64+z1, z0, kh, kw]; bias in wt[1][96]
    Wr = np.asarray(W, np.float32).reshape(T1, Z1, 32, 3, 3)
    wt = np.zeros((3, K, 512), np.float32)
    # -> [kw, kh, z0, z1, t1]
    Wp = np.transpose(Wr, (4, 3, 2, 1, 0))
    wt[:, :96, :] = Wp.reshape(3, 96, 512)
    bz = np.asarray(bias, np.float32).reshape(T1, Z1).T.reshape(512)  # z1*8+t1
    wt[1, 96, :] = bz
    return wt


def _consts():
    ident = np.eye(128, dtype=np.float16)
    ind8 = np.zeros((64, T0), np.float32)
    for p in range(64):
        ind8[p, p // T1] = 1.0
    ind8t = np.zeros((T0, 64), np.float32)
    for m in range(64):
        ind8t[m // T1, m] = 1.0
    import ml_dtypes

    return ident, ind8.astype(ml_dtypes.bfloat16), ind8t.astype(np.float32)


def _run(inputs, trace=False, **kw):
    x = np.ascontiguousarray(np.asarray(inputs["x"], np.float32))
    wt = _host_prep(inputs["W"], inputs["bias"])
    ident, ind8, ind8t = _consts()
    nc = _get_nc()
    in_maps = [
        {"x": x[i], "wt": wt, "ident": ident, "ind8": ind8, "ind8t": ind8t}
        for i in range(8)
    ]
    res = run_bass_kernel_spmd(nc, in_maps, core_ids=list(range(8)), trace=trace, **kw)
    full = np.stack([res.results[i]["out"] for i in range(8)], axis=0)
    return full, res


def kernel(**inputs):
    full, _ = _run(inputs)
    return full


if __name__ == "__main__":
    rng = np.random.default_rng(0)
    ins = {
        "x": rng.normal(size=(8, 8, 32, 64, 64)).astype(np.float32),
        "W": (rng.normal(size=(512, 32, 3, 3)) * 0.05).astype(np.float32),
        "bias": (rng.normal(size=(512,)) * 0.01).astype(np.float32),
    }
    out = kernel(**ins)
    print(out.shape, out.dtype)

